# revision 1
# baseline (speedup 1.0000x reference)
"""Trainium2 Bass kernel for the AgentLoss problem (raw bacc, manual sems).

Math: for each (l, b) the reference computes the masked cosine-similarity sum
    S = sum_{i != j} <x_i, x_j> / (|x_i| |x_j| + EPS)
over n=1024 agents with c=64 channels, then loss = sum_l mean_b S / (n(n-1)).

Since EPS (1e-5) is tiny vs |x_i||x_j| ~ 64, expand
    1/(m_i m_j + EPS) = r_i r_j - EPS r_i^2 r_j^2 + O(EPS^2),  r_i = 1/m_i
which makes the double sum separable:
    S ~= (|sum_i x_i r_i|^2 - sum_i msq_i r_i^2)
         - EPS * (|sum_i x_i r_i^2|^2 - sum_i msq_i r_i^4)
(order-1 truncation error ~3e-14 relative - validated vs fp64).

Device work per (l, b) pair: row norms (square + segmented reduce, squares
split ACT/GpSimd to balance engines), r^2 = DVE reciprocal of msq written
straight into the weight tile, r = ACT sqrt of it, then thin fp32 matmuls
contracting the agent axis with [r, r^2] weight columns, packed two sub-tiles
per matmul (N=128, half-garbage outputs the host discards). The diagonal
corrections collapse: sum_i msq_i r_i^2 = n to fp32 rounding (~1e-7), and
sum_i msq_i r_i^4 ~= sum_i r_i^2, computed by a ones-matmul over the r^2
columns. Groups are sized (2,2,2,1,1) so the final dependency chain drains
through a single pair. A dummy sqrt up front pulls the ACT table load off
the critical path. Host does the final ~2k-flop combine in float64.

Sharding: data-parallel over batch b - core k takes b in {2k, 2k+1}, i.e.
8 (l, b_local) pairs per core. Each core returns a [4, 1088] block.
"""

from contextlib import ExitStack

import numpy as np

import concourse.bass as bass
from concourse import bacc, mybir
from concourse.bass_utils import run_bass_kernel_spmd

EPS = 1e-5
L, B, N, C = 4, 16, 1024, 64
P = 128            # SBUF partitions
T = N // P         # 8 agent sub-rows per partition
NCORES = 8
BPC = B // NCORES  # b per core
NPAIR = L * BPC    # (l, b_local) pairs per core
GROUPS = [[0, 1], [2, 3], [4, 5], [6], [7]]  # pairs per norm/weights group
NG = len(GROUPS)
GP_SQ_PAIRS = (1, 3, 5)  # early odd pairs' squares on GpSimd

F32 = mybir.dt.float32
OUT_W = NPAIR * P + NPAIR * 8  # 1024 + 64
PQ_OFF = [8 * sum(len(GROUPS[h]) for h in range(g)) for g in range(NG)]


def build_nc() -> bass.Bass:
    nc = bacc.Bacc("TRN2", target_bir_lowering=False, debug=False, num_devices=NCORES)
    x = nc.declare_dram_parameter("x", [NPAIR, N, C], F32, isOutput=False)
    out = nc.declare_dram_parameter("out", [4, OUT_W], F32, isOutput=True)

    ctx = ExitStack()
    with ctx:
        def sb(name, shape):
            return ctx.enter_context(nc.sbuf_tensor(name, shape, F32))

        xp = [sb(f"xp{j}", [P, T, C]) for j in range(NPAIR)]
        xsq = [sb(f"xsq{j}", [P, T, C]) for j in range(NPAIR)]
        msq = [sb(f"msq{g}", [P, 8 * len(GROUPS[g])]) for g in range(NG)]
        RR = [sb(f"RR{g}", [P, 16 * len(GROUPS[g])]) for g in range(NG)]
        ones = sb("ones", [P, 1])
        scr_s = sb("scr_s", [P, 1])
        stage = sb("stage", [4, OUT_W])
        psum_s = ctx.enter_context(nc.psum_tensor("psum_s", [4, NPAIR * 256], F32))
        psum_pq = ctx.enter_context(nc.psum_tensor("psum_pq", [1, NPAIR * 8], F32))

        s_dma = [nc.alloc_semaphore(f"s_dma{j}") for j in range(NPAIR)]
        s_dmo = nc.alloc_semaphore("s_dmo")
        s_z = nc.alloc_semaphore("s_z")        # ones ready
        s_act = nc.alloc_semaphore("s_act")    # ACT squares done (ordered)
        s_actg = nc.alloc_semaphore("s_actg")  # GpSimd squares done (ordered)
        s_inv = nc.alloc_semaphore("s_inv")    # reciprocal done (per group)
        s_rr = nc.alloc_semaphore("s_rr")      # weights ready (per group)
        s_pe = nc.alloc_semaphore("s_pe")      # matmul progress (1..5)
        s_stage = nc.alloc_semaphore("s_stage")   # DVE staging copies (1..5)
        s_dve = nc.alloc_semaphore("s_dve")    # DVE same-engine RAW chain
        sems = s_dma + [s_dmo, s_z, s_act, s_actg, s_inv, s_rr, s_pe,
                        s_stage, s_dve]

        ACT_SQ = [j for j in range(NPAIR) if j not in GP_SQ_PAIRS]

        def rv4(g, lo, hi):
            # view RR[g]'s (slot, tt, f=4) layout, f-slice [lo:hi)
            return RR[g][:].rearrange(
                "p (s tt f) -> p s tt f", s=len(GROUPS[g]), tt=4
            )[:, :, :, lo:hi]

        def mq4(g):
            return msq[g][:].rearrange(
                "p (s tt f) -> p s tt f", s=len(GROUPS[g]), tt=4
            )

        with nc.Block() as block:

            @block.sync
            def _(sync):
                for j in range(NPAIR):
                    sync.dma_start(
                        out=xp[j][:], in_=x[j].rearrange("(p t) c -> p t c", p=P)
                    ).then_inc(s_dma[j], 16)
                sync.wait_ge(s_stage, 2)
                sync.dma_start(out=out[:, 0:512], in_=stage[:, 0:512]).then_inc(
                    s_dmo, 16
                )
                sync.wait_ge(s_stage, 5)
                sync.dma_start(
                    out=out[:, 512:OUT_W], in_=stage[:, 512:OUT_W]
                ).then_inc(s_dmo, 16)

            @block.scalar
            def _(scalar):
                # dummy sqrt: pulls the sqrt+square ACT table load into the
                # DMA phase
                scalar.sqrt(scr_s[:], ones[:])._wait_ge(s_z, 1)

                def norm_group(g):
                    # r = sqrt(r^2): reads the r^2 (=1/msq) columns the DVE
                    # reciprocal wrote, fills the r columns
                    scalar.activation(
                        rv4(g, 0, 2),
                        rv4(g, 2, 4),
                        mybir.ActivationFunctionType.Sqrt,
                    )._wait_ge(s_inv, g + 1).then_inc(s_rr)

                def sq(j):
                    scalar.square(xsq[j][:], xp[j][:])._wait_ge(
                        s_dma[j], 16
                    ).then_inc(s_act)

                sq(0)
                sq(2)
                norm_group(0)
                sq(4)
                norm_group(1)
                sq(6)
                norm_group(2)
                sq(7)
                norm_group(3)
                norm_group(4)

            @block.vector
            def _(vector):
                vector.memset(stage[:, NPAIR * P : OUT_W], 0.0)
                vd = [0]
                for g, pairs in enumerate(GROUPS):
                    for slot, j in enumerate(pairs):
                        red = vector.tensor_reduce(
                            out=msq[g][:, slot * 8 : slot * 8 + 8],
                            in_=xsq[j][:],
                            axis=mybir.AxisListType.X,
                            op=mybir.AluOpType.add,
                        )
                        if j in GP_SQ_PAIRS:
                            red._wait_ge(s_actg, GP_SQ_PAIRS.index(j) + 1)
                        else:
                            red._wait_ge(s_act, ACT_SQ.index(j) + 1)
                        red.then_inc(s_dve)
                        vd[0] += 1
                    # r^2 = 1/msq straight into the weight tile
                    vector.reciprocal(
                        out=rv4(g, 2, 4), in_=mq4(g)
                    )._wait_ge(s_dve, vd[0]).then_inc(s_inv)
                # bank b holds pairs 2b, 2b+1; wait until the owning groups'
                # matmuls are done (s_pe counts groups)
                bank_pe = [1, 2, 3, 5]
                psv = psum_s[:].rearrange("p (j c) -> p j c", c=256)
                for b in range(4):
                    vector.tensor_copy(
                        stage[:, 256 * b : 256 * (b + 1)].rearrange(
                            "p (j c) -> p j c", c=P
                        ),
                        psv[:, 2 * b : 2 * b + 2, 0:P],
                    )._wait_ge(s_pe, bank_pe[b]).then_inc(s_stage)
                vector.tensor_copy(
                    stage[0:1, NPAIR * P : OUT_W], psum_pq[:]
                )._wait_ge(s_pe, 6).then_inc(s_stage)

            @block.gpsimd
            def _(gpsimd):
                gpsimd.memset(ones[:], 1.0).then_inc(s_z)
                for j in GP_SQ_PAIRS:
                    gpsimd.tensor_mul(xsq[j][:], xp[j][:], xp[j][:])._wait_ge(
                        s_dma[j], 16
                    ).then_inc(s_actg)

            @block.tensor
            def _(tensor):
                tensor.wait_ge(s_z, 1)

                def pq_mm(g, inc_pe=False):
                    # t2 partial sums: ones^T @ r^2 columns
                    mm = tensor.matmul(
                        psum_pq[:, PQ_OFF[g] : PQ_OFF[g] + 8 * len(GROUPS[g])],
                        ones[:],
                        rv4(g, 2, 4),
                        start=True,
                        stop=True,
                    )
                    if inc_pe:
                        mm.then_inc(s_pe)

                for g, pairs in enumerate(GROUPS):
                    tensor.wait_ge(s_rr, g + 1)
                    for slot, j in enumerate(pairs):
                        tensor.wait_ge(s_dma[j], 16)
                        for tt in range(T // 2):
                            mm = tensor.matmul(
                                psum_s[:, 256 * j : 256 * j + P],
                                RR[g][:, slot * 16 + tt * 4 : slot * 16 + tt * 4 + 4],
                                xp[j][:, 2 * tt : 2 * tt + 2, :],
                                start=(tt == 0),
                                stop=(tt == T // 2 - 1),
                            )
                            if slot == len(pairs) - 1 and tt == T // 2 - 1:
                                mm.then_inc(s_pe)
                    # the group's own s_rr wait already covers its r^2 columns
                    pq_mm(g, inc_pe=(g == NG - 1))

        # the out-DMA write receipt (~2.4us) elapses during the block-exit
        # barrier above; the wait still precedes stream end and the clears,
        # so the host read and re-execution stay safe
        nc.sync.wait_ge(s_dmo, 32)
        for s in sems:
            nc.sync.sem_clear(s)

    nc.compile()
    return nc


_NC_CACHE = None


def _get_nc():
    global _NC_CACHE
    if _NC_CACHE is None:
        _NC_CACHE = build_nc()
    return _NC_CACHE


def run_cores(x_full: np.ndarray, trace: bool = False):
    """Shard, run on 8 NeuronCores, return (per-core out blocks, results obj)."""
    nc = _get_nc()
    in_maps = []
    for k in range(NCORES):
        shard = np.ascontiguousarray(
            x_full[:, BPC * k : BPC * (k + 1)].reshape(NPAIR, N, C)
        )
        in_maps.append({"x": shard})
    res = run_bass_kernel_spmd(nc, in_maps, list(range(NCORES)), trace=trace)
    outs = [res.results[k]["out"] for k in range(NCORES)]
    return outs, res


def _group_of(j):
    for g, pairs in enumerate(GROUPS):
        if j in pairs:
            return g, pairs.index(j)
    raise ValueError(j)


def reduce_host(outs) -> np.ndarray:
    total = 0.0
    for blk in outs:
        blk = blk.astype(np.float64)
        for j in range(NPAIR):
            g, slot = _group_of(j)
            s = blk[0, P * j : P * j + 64] + blk[1, P * j + 64 : P * j + 128]
            s2 = blk[2, P * j : P * j + 64] + blk[3, P * j + 64 : P * j + 128]
            base = NPAIR * P + PQ_OFF[g] + slot * 8
            t2_sum = blk[0, base : base + 8].sum()
            S0 = np.dot(s, s) - float(N)
            S1 = np.dot(s2, s2) - t2_sum
            total += S0 - EPS * S1
    loss = total / (N * (N - 1)) / B
    return np.array(loss, dtype=np.float32)


def kernel(updated_agents: np.ndarray) -> np.ndarray:
    outs, _ = run_cores(np.asarray(updated_agents))
    return reduce_host(outs)



# revision 15
# speedup vs baseline: 1.1395x; 1.1395x over previous
"""Trainium2 Bass kernel for the AgentLoss problem (raw bacc, manual sems).

Math: for each (l, b) the reference computes the masked cosine-similarity sum
    S = sum_{i != j} <x_i, x_j> / (|x_i| |x_j| + EPS)
over n=1024 agents with c=64 channels, then loss = sum_l mean_b S / (n(n-1)).

Since EPS (1e-5) is tiny vs |x_i||x_j| ~ 64, expand
    1/(m_i m_j + EPS) = r_i r_j - EPS r_i^2 r_j^2 + O(EPS^2),  r_i = 1/m_i
which makes the double sum separable:
    S ~= (|sum_i x_i r_i|^2 - sum_i msq_i r_i^2)
         - EPS * (|sum_i x_i r_i^2|^2 - sum_i msq_i r_i^4)
with sum_i msq_i r_i^2 ~= n and sum_i msq_i r_i^4 ~= sum_i r_i^2 (= t2).

This version runs the whole device side in bf16: the host pre-casts the
input (cosine similarity is scale-free and smoothly averaged, so the cast
costs ~1e-4 relative error), which halves HBM traffic and lets the PE
stream the matmuls at full rate instead of fp32's LOW/HIGH half-rate
split.  Per (l, b) pair: square (ACT / GpSimd), segmented reduce to per-
agent msq (DVE), r^2 = 1/msq (DVE reciprocal), then ACT writes the bf16
weight tile (sqrt for r, copy-cast for r^2).  Thin bf16 matmuls contract
the agent axis, packing 2 sub-rows x {r, r^2} per matmul (N=128 moving,
half-garbage outputs the host discards).  t2 comes from a ones-stationary
matmul over the r^2 weight columns.  Host does the final ~2k-flop combine
in float64.

Sharding: data-parallel over batch b - core k takes b in {2k, 2k+1}, i.e.
8 (l, b_local) pairs per core. Each core returns a [4, 1088] block.
"""

from contextlib import ExitStack

import numpy as np
import ml_dtypes

import concourse.bass as bass
from concourse import bacc, mybir
from concourse.bass_utils import run_bass_kernel_spmd

EPS = 1e-5
L, B, N, C = 4, 16, 1024, 64
P = 128            # SBUF partitions
T = N // P         # 8 agent sub-rows per partition
NCORES = 8
BPC = B // NCORES  # b per core
NPAIR = L * BPC    # (l, b_local) pairs per core

DMA_CHUNKS = [(0, 2), (2, 5), (5, 8)]          # j-ranges per input DMA
GROUPS = [[0, 1], [2, 3], [4, 5], [6], [7]]    # pairs per recip/weights group
NG = len(GROUPS)
ACT_SQ = (0, 2, 3, 5, 7)   # squares on ACT
GP_SQ = (1, 4, 6)          # squares on GpSimd

F32 = mybir.dt.float32
BF16 = mybir.dt.bfloat16
OUT_W = NPAIR * P + NPAIR * 8  # 1024 + 64


def _chunk_of(j):
    for k, (a, b) in enumerate(DMA_CHUNKS):
        if a <= j < b:
            return k
    raise ValueError(j)


def _group_of(j):
    for g, pairs in enumerate(GROUPS):
        if j in pairs:
            return g, pairs.index(j)
    raise ValueError(j)


def build_nc() -> bass.Bass:
    nc = bacc.Bacc("TRN2", target_bir_lowering=False, debug=False, num_devices=NCORES)
    x = nc.declare_dram_parameter("x", [P, NPAIR, T, C], BF16, isOutput=False)
    out = nc.declare_dram_parameter("out", [4, OUT_W], F32, isOutput=True)

    one_f32 = nc.const_aps.aps[(F32, 1.0)]
    one_bf16 = nc.const_aps.aps[(BF16, 1.0)]

    ctx = ExitStack()
    with ctx:
        def sb(name, shape, dtype=F32):
            return ctx.enter_context(nc.sbuf_tensor(name, shape, dtype))

        xb = sb("xb", [P, NPAIR, T, C], BF16)
        xsq = sb("xsq", [P, NPAIR, T, C])
        msq = sb("msq", [P, NPAIR, T])
        rsq = sb("rsq", [P, NPAIR, T])
        W = sb("W", [P, NPAIR, 4, 4], BF16)   # (tt, [r,r,r2,r2])
        scr = sb("scr", [P, 1])
        stage = sb("stage", [4, OUT_W])
        psum_s = [
            ctx.enter_context(nc.psum_tensor(f"psum_s{h}", [4, 2 * P], F32))
            for h in range(4)
        ]
        psum_pq = ctx.enter_context(nc.psum_tensor("psum_pq", [1, NPAIR * 8], F32))

        s_dma = [nc.alloc_semaphore(f"s_dma{k}") for k in range(len(DMA_CHUNKS))]
        s_sqa = nc.alloc_semaphore("s_sqa")    # ACT squares done (ordered)
        s_sqg = nc.alloc_semaphore("s_sqg")    # GpSimd squares done (ordered)
        s_rsq = nc.alloc_semaphore("s_rsq")    # DVE reciprocal done (per group)
        s_w = nc.alloc_semaphore("s_w")        # bf16 weights ready (per group)
        s_pe = nc.alloc_semaphore("s_pe")      # matmul progress (1..5)
        s_st = nc.alloc_semaphore("s_st")      # DVE staging copies (1..3)
        s_sta = nc.alloc_semaphore("s_sta")    # ACT staging copies (1..2)
        s_dmo = nc.alloc_semaphore("s_dmo")    # out DMA receipts
        s_dve = nc.alloc_semaphore("s_dve")    # DVE same-engine RAW chain
        sems = s_dma + [s_sqa, s_sqg, s_rsq, s_w, s_pe, s_st, s_sta, s_dmo,
                        s_dve]

        with nc.Block() as block:

            @block.sync
            def _(sync):
                for k, (a, b) in enumerate(DMA_CHUNKS):
                    sync.dma_start(
                        out=xb[:, a:b], in_=x[:, a:b]
                    ).then_inc(s_dma[k], 16)
                sync.wait_ge(s_st, 1)
                sync.wait_ge(s_sta, 1)
                sync.dma_start(out=out[:, 0:512], in_=stage[:, 0:512]).then_inc(
                    s_dmo, 16
                )
                sync.wait_ge(s_st, 3)
                sync.wait_ge(s_sta, 2)
                sync.wait_ge(s_sqg, 1)
                sync.dma_start(
                    out=out[:, 512:OUT_W], in_=stage[:, 512:OUT_W]
                ).then_inc(s_dmo, 16)

            @block.scalar
            def _(scalar):
                # dummy sqrt pulls the ACT table load off the critical path
                scalar.sqrt(scr[:], one_f32)

                def sq(j):
                    scalar.square(xsq[:, j], xb[:, j])._wait_ge(
                        s_dma[_chunk_of(j)], 16
                    ).then_inc(s_sqa)

                def weights(g):
                    pairs = GROUPS[g]
                    a, b = pairs[0], pairs[-1] + 1
                    rv = rsq[:, a:b].rearrange("p j (tt u) -> p j tt u", u=2)
                    scalar.activation(
                        W[:, a:b, :, 0:2],
                        rv,
                        mybir.ActivationFunctionType.Sqrt,
                    )._wait_ge(s_rsq, g + 1)
                    scalar.copy(
                        W[:, a:b, :, 2:4],
                        rv,
                    ).then_inc(s_w)

                sq(0)
                sq(2)
                sq(3)
                weights(0)
                sq(5)
                weights(1)
                sq(7)
                weights(2)
                weights(3)
                weights(4)
                # staging copies for pairs 2-3 and 6-7
                scalar.copy(
                    stage[:, 256:512], psum_s[1][:]
                )._wait_ge(s_pe, 2).then_inc(s_sta)
                scalar.copy(
                    stage[:, 768:1024], psum_s[3][:]
                )._wait_ge(s_pe, 4).then_inc(s_sta)

            @block.gpsimd
            def _(gpsimd):
                # rows 1-3 of the pq slot are never written; zero them so the
                # out-DMA reads defined bytes
                gpsimd.memset(stage[:, NPAIR * P : OUT_W], 0.0).then_inc(s_sqg)
                for j in GP_SQ:
                    gpsimd.tensor_mul(xsq[:, j], xb[:, j], xb[:, j])._wait_ge(
                        s_dma[_chunk_of(j)], 16
                    ).then_inc(s_sqg)

            @block.vector
            def _(vector):
                nred = [0]

                def red(j):
                    r = vector.tensor_reduce(
                        out=msq[:, j],
                        in_=xsq[:, j],
                        axis=mybir.AxisListType.X,
                        op=mybir.AluOpType.add,
                    )
                    if j in GP_SQ:
                        r._wait_ge(s_sqg, GP_SQ.index(j) + 2)
                    else:
                        r._wait_ge(s_sqa, ACT_SQ.index(j) + 1)
                    r.then_inc(s_dve)
                    nred[0] += 1

                def recip(g):
                    pairs = GROUPS[g]
                    a, b = pairs[0], pairs[-1] + 1
                    vector.reciprocal(out=rsq[:, a:b], in_=msq[:, a:b])._wait_ge(
                        s_dve, nred[0]
                    ).then_inc(s_rsq)

                red(0)
                red(1)
                recip(0)
                red(2)
                red(3)
                recip(1)
                red(4)
                red(5)
                recip(2)
                red(6)
                recip(3)
                red(7)
                recip(4)
                # staging copies for pairs 0-1, 4-5 and the pq row
                vector.tensor_copy(
                    stage[:, 0:256], psum_s[0][:]
                )._wait_ge(s_pe, 1).then_inc(s_st)
                vector.tensor_copy(
                    stage[:, 512:768], psum_s[2][:]
                )._wait_ge(s_pe, 3).then_inc(s_st)
                vector.wait_ge(s_sqg, 1)
                vector.tensor_copy(
                    stage[0:1, NPAIR * P : OUT_W], psum_pq[:]
                )._wait_ge(s_pe, 5).then_inc(s_st)

            @block.tensor
            def _(tensor):
                for j in range(NPAIR):
                    g, _slot = _group_of(j)
                    tensor.wait_ge(s_w, g + 1)
                    tensor.wait_ge(s_dma[_chunk_of(j)], 16)
                    for tt in range(T // 2):
                        mm = tensor.matmul(
                            psum_s[j // 2][:, P * (j % 2) : P * (j % 2) + P],
                            W[:, j, tt],
                            xb[:, j, 2 * tt : 2 * tt + 2, :],
                            start=(tt == 0),
                            stop=(tt == T // 2 - 1),
                        )
                        if j % 2 == 1 and tt == T // 2 - 1:
                            mm.then_inc(s_pe)
                # t2 partial sums: ones^T @ r^2 columns (weights all ready
                # because pair 7's s-matmuls waited on the last group)
                for j in range(NPAIR):
                    mm = tensor.matmul(
                        psum_pq[:, 8 * j : 8 * j + 8],
                        one_bf16,
                        W[:, j, :, 2:4],
                        start=True,
                        stop=True,
                    )
                    if j == NPAIR - 1:
                        mm.then_inc(s_pe)

        # out-DMA receipt elapses mostly during the block-exit barrier
        nc.sync.wait_ge(s_dmo, 32)
        for s in sems:
            nc.sync.sem_clear(s)

    nc.compile()
    return nc


_NC_CACHE = None


def _get_nc():
    global _NC_CACHE
    if _NC_CACHE is None:
        _NC_CACHE = build_nc()
    return _NC_CACHE


def _shard_inputs(x_full: np.ndarray):
    """Full [L, B, N, C] fp32 -> per-core [P, NPAIR, T, C] bf16 blocks."""
    in_maps = []
    for k in range(NCORES):
        shard = x_full[:, BPC * k : BPC * (k + 1)].reshape(NPAIR, P, T, C)
        shard = np.ascontiguousarray(shard.transpose(1, 0, 2, 3)).astype(
            ml_dtypes.bfloat16
        )
        in_maps.append({"x": shard})
    return in_maps


def run_cores(x_full: np.ndarray, trace: bool = False):
    nc = _get_nc()
    in_maps = _shard_inputs(np.asarray(x_full))
    res = run_bass_kernel_spmd(nc, in_maps, list(range(NCORES)), trace=trace)
    outs = [res.results[k]["out"] for k in range(NCORES)]
    return outs, res


def reduce_host(outs) -> np.ndarray:
    total = 0.0
    for blk in outs:
        blk = blk.astype(np.float64)
        for j in range(NPAIR):
            s = blk[0, P * j : P * j + 64] + blk[1, P * j + 64 : P * j + 128]
            s2 = blk[2, P * j : P * j + 64] + blk[3, P * j + 64 : P * j + 128]
            t2 = blk[0, NPAIR * P + 8 * j : NPAIR * P + 8 * j + 8].sum()
            S0 = np.dot(s, s) - float(N)
            S1 = np.dot(s2, s2) - t2
            total += S0 - EPS * S1
    loss = total / (N * (N - 1)) / B
    return np.array(loss, dtype=np.float32)


def kernel(updated_agents: np.ndarray) -> np.ndarray:
    outs, _ = run_cores(np.asarray(updated_agents))
    return reduce_host(outs)


# revision 18
# speedup vs baseline: 1.1546x; 1.0133x over previous
"""Trainium2 Bass kernel for the AgentLoss problem (raw bacc, manual sems).

Math: for each (l, b) the reference computes the masked cosine-similarity sum
    S = sum_{i != j} <x_i, x_j> / (|x_i| |x_j| + EPS)
over n=1024 agents with c=64 channels, then loss = sum_l mean_b S / (n(n-1)).

Since EPS (1e-5) is tiny vs |x_i||x_j| ~ 64, expand
    1/(m_i m_j + EPS) = r_i r_j - EPS r_i^2 r_j^2 + O(EPS^2),  r_i = 1/m_i
which makes the double sum separable:
    S ~= (|sum_i x_i r_i|^2 - sum_i msq_i r_i^2)
         - EPS * (|sum_i x_i r_i^2|^2 - sum_i msq_i r_i^4)
with sum_i msq_i r_i^2 ~= n and sum_i msq_i r_i^4 ~= sum_i r_i^2 (= t2).

This version runs the whole device side in bf16: the host pre-casts the
input (cosine similarity is scale-free and smoothly averaged, so the cast
costs ~1e-4 relative error), which halves HBM traffic and lets the PE
stream the matmuls at full rate instead of fp32's LOW/HIGH half-rate
split.  Per (l, b) pair: square (ACT / GpSimd), segmented reduce to per-
agent msq (DVE), r^2 = 1/msq (DVE reciprocal), then ACT writes the bf16
weight tile (sqrt for r, copy-cast for r^2).  Thin bf16 matmuls contract
the agent axis, packing 2 sub-rows x {r, r^2} per matmul (N=128 moving,
half-garbage outputs the host discards).  t2 comes from a ones-stationary
matmul over the r^2 weight columns.  Host does the final ~2k-flop combine
in float64.

Sharding: data-parallel over batch b - core k takes b in {2k, 2k+1}, i.e.
8 (l, b_local) pairs per core. Each core returns a [4, 1088] block.
"""

from contextlib import ExitStack

import numpy as np
import ml_dtypes

import concourse.bass as bass
from concourse import bacc, mybir
from concourse.bass_utils import run_bass_kernel_spmd

EPS = 1e-5
L, B, N, C = 4, 16, 1024, 64
P = 128            # SBUF partitions
T = N // P         # 8 agent sub-rows per partition
NCORES = 8
BPC = B // NCORES  # b per core
NPAIR = L * BPC    # (l, b_local) pairs per core

DMA_CHUNKS = [(0, 1), (1, 3), (3, 5), (5, 8)]  # j-ranges per input DMA
GROUPS = [[0, 1], [2, 3], [4, 5], [6], [7]]    # pairs per recip/weights group
NG = len(GROUPS)
ACT_SQ = (0, 2, 3, 5, 7)   # squares on ACT
GP_SQ = (1, 4, 6)          # squares on GpSimd

F32 = mybir.dt.float32
BF16 = mybir.dt.bfloat16
OUT_W = NPAIR * P + NPAIR * 8  # 1024 + 64


def _chunk_of(j):
    for k, (a, b) in enumerate(DMA_CHUNKS):
        if a <= j < b:
            return k
    raise ValueError(j)


def _group_of(j):
    for g, pairs in enumerate(GROUPS):
        if j in pairs:
            return g, pairs.index(j)
    raise ValueError(j)


def build_nc() -> bass.Bass:
    nc = bacc.Bacc("TRN2", target_bir_lowering=False, debug=False, num_devices=NCORES)
    x = nc.declare_dram_parameter("x", [P, NPAIR, T, C], BF16, isOutput=False)
    out = nc.declare_dram_parameter("out", [4, OUT_W], F32, isOutput=True)

    one_f32 = nc.const_aps.aps[(F32, 1.0)]
    one_bf16 = nc.const_aps.aps[(BF16, 1.0)]

    ctx = ExitStack()
    with ctx:
        def sb(name, shape, dtype=F32):
            return ctx.enter_context(nc.sbuf_tensor(name, shape, dtype))

        xb = sb("xb", [P, NPAIR, T, C], BF16)
        xsq = sb("xsq", [P, NPAIR, T, C])
        msq = sb("msq", [P, NPAIR, T])
        rsq = sb("rsq", [P, NPAIR, T])
        W = sb("W", [P, NPAIR, 4, 4], BF16)   # (tt, [r,r,r2,r2])
        scr = sb("scr", [P, 1])
        stage = sb("stage", [4, OUT_W])
        psum_s = [
            ctx.enter_context(nc.psum_tensor(f"psum_s{h}", [4, 2 * P], F32))
            for h in range(4)
        ]
        psum_pq = ctx.enter_context(nc.psum_tensor("psum_pq", [1, NPAIR * 8], F32))

        s_dma = [nc.alloc_semaphore(f"s_dma{k}") for k in range(len(DMA_CHUNKS))]
        s_sqa = nc.alloc_semaphore("s_sqa")    # ACT squares done (ordered)
        s_sqg = nc.alloc_semaphore("s_sqg")    # GpSimd squares done (ordered)
        s_rsq = nc.alloc_semaphore("s_rsq")    # DVE reciprocal done (per group)
        s_w = nc.alloc_semaphore("s_w")        # r weights ready (per group)
        s_w2 = nc.alloc_semaphore("s_w2")      # r^2 weights ready (per group)
        s_pe = nc.alloc_semaphore("s_pe")      # matmul progress (1..5)
        s_st = nc.alloc_semaphore("s_st")      # DVE staging copies (1..3)
        s_sta = nc.alloc_semaphore("s_sta")    # ACT staging copies (1..2)
        s_dmo = nc.alloc_semaphore("s_dmo")    # out DMA receipts
        s_dve = nc.alloc_semaphore("s_dve")    # DVE same-engine RAW chain
        sems = s_dma + [s_sqa, s_sqg, s_rsq, s_w, s_w2, s_pe, s_st, s_sta,
                        s_dmo, s_dve]

        with nc.Block() as block:

            @block.sync
            def _(sync):
                for k, (a, b) in enumerate(DMA_CHUNKS):
                    sync.dma_start(
                        out=xb[:, a:b], in_=x[:, a:b]
                    ).then_inc(s_dma[k], 16)
                sync.wait_ge(s_st, 1)
                sync.wait_ge(s_sta, 1)
                sync.dma_start(out=out[:, 0:512], in_=stage[:, 0:512]).then_inc(
                    s_dmo, 16
                )
                sync.wait_ge(s_st, 3)
                sync.wait_ge(s_sta, 2)
                sync.wait_ge(s_sqg, 1)
                sync.dma_start(
                    out=out[:, 512:OUT_W], in_=stage[:, 512:OUT_W]
                ).then_inc(s_dmo, 16)

            @block.scalar
            def _(scalar):
                # dummy sqrt pulls the ACT table load off the critical path
                scalar.sqrt(scr[:], one_f32)

                def sq(j):
                    scalar.square(xsq[:, j], xb[:, j])._wait_ge(
                        s_dma[_chunk_of(j)], 16
                    ).then_inc(s_sqa)

                def weights(g):
                    pairs = GROUPS[g]
                    a, b = pairs[0], pairs[-1] + 1
                    scalar.activation(
                        W[:, a:b, :, 0:2],
                        rsq[:, a:b].rearrange("p j (tt u) -> p j tt u", u=2),
                        mybir.ActivationFunctionType.Sqrt,
                    )._wait_ge(s_rsq, g + 1).then_inc(s_w)

                sq(0)
                sq(2)
                sq(3)
                weights(0)
                sq(5)
                weights(1)
                sq(7)
                weights(2)
                weights(3)
                weights(4)
                # staging copies for pairs 2-3 and 6-7
                scalar.copy(
                    stage[:, 256:512], psum_s[1][:]
                )._wait_ge(s_pe, 2).then_inc(s_sta)
                scalar.copy(
                    stage[:, 768:1024], psum_s[3][:]
                )._wait_ge(s_pe, 4).then_inc(s_sta)

            @block.gpsimd
            def _(gpsimd):
                # rows 1-3 of the pq slot are never written; zero them so the
                # out-DMA reads defined bytes
                gpsimd.memset(stage[:, NPAIR * P : OUT_W], 0.0).then_inc(s_sqg)

                def sq(j):
                    gpsimd.tensor_mul(xsq[:, j], xb[:, j], xb[:, j])._wait_ge(
                        s_dma[_chunk_of(j)], 16
                    ).then_inc(s_sqg)

                def w2(g):
                    pairs = GROUPS[g]
                    a, b = pairs[0], pairs[-1] + 1
                    gpsimd.tensor_copy(
                        W[:, a:b, :, 2:4],
                        rsq[:, a:b].rearrange("p j (tt u) -> p j tt u", u=2),
                    )._wait_ge(s_rsq, g + 1).then_inc(s_w2)

                sq(1)
                w2(0)
                sq(4)
                w2(1)
                sq(6)
                w2(2)
                w2(3)
                w2(4)

            @block.vector
            def _(vector):
                nred = [0]

                def red(j):
                    r = vector.tensor_reduce(
                        out=msq[:, j],
                        in_=xsq[:, j],
                        axis=mybir.AxisListType.X,
                        op=mybir.AluOpType.add,
                    )
                    if j in GP_SQ:
                        r._wait_ge(s_sqg, GP_SQ.index(j) + 2)
                    else:
                        r._wait_ge(s_sqa, ACT_SQ.index(j) + 1)
                    r.then_inc(s_dve)
                    nred[0] += 1

                def recip(g):
                    pairs = GROUPS[g]
                    a, b = pairs[0], pairs[-1] + 1
                    vector.reciprocal(out=rsq[:, a:b], in_=msq[:, a:b])._wait_ge(
                        s_dve, nred[0]
                    ).then_inc(s_rsq)

                red(0)
                red(1)
                recip(0)
                red(2)
                red(3)
                recip(1)
                red(4)
                red(5)
                recip(2)
                red(6)
                recip(3)
                red(7)
                recip(4)
                # staging copies for pairs 0-1, 4-5 and the pq row
                vector.tensor_copy(
                    stage[:, 0:256], psum_s[0][:]
                )._wait_ge(s_pe, 1).then_inc(s_st)
                vector.tensor_copy(
                    stage[:, 512:768], psum_s[2][:]
                )._wait_ge(s_pe, 3).then_inc(s_st)
                vector.wait_ge(s_sqg, 1)
                vector.tensor_copy(
                    stage[0:1, NPAIR * P : OUT_W], psum_pq[:]
                )._wait_ge(s_pe, 5).then_inc(s_st)

            @block.tensor
            def _(tensor):
                def smm(j, inc=False):
                    g, _slot = _group_of(j)
                    tensor.wait_ge(s_w, g + 1)
                    tensor.wait_ge(s_w2, g + 1)
                    tensor.wait_ge(s_dma[_chunk_of(j)], 16)
                    for tt in range(T // 2):
                        mm = tensor.matmul(
                            psum_s[j // 2][:, P * (j % 2) : P * (j % 2) + P],
                            W[:, j, tt],
                            xb[:, j, 2 * tt : 2 * tt + 2, :],
                            start=(tt == 0),
                            stop=(tt == T // 2 - 1),
                        )
                        if inc and tt == T // 2 - 1:
                            mm.then_inc(s_pe)

                def pq(j, inc=False):
                    # t2 partial sums: ones^T @ r^2 columns
                    mm = tensor.matmul(
                        psum_pq[:, 8 * j : 8 * j + 8],
                        one_bf16,
                        W[:, j, :, 2:4],
                        start=True,
                        stop=True,
                    )
                    if inc:
                        mm.then_inc(s_pe)

                for j in range(7):
                    smm(j, inc=(j in (1, 3, 5)))
                for j in range(7):
                    pq(j)
                smm(7, inc=True)
                pq(7, inc=True)

        # out-DMA receipt elapses mostly during the block-exit barrier
        nc.sync.wait_ge(s_dmo, 32)
        for s in sems:
            nc.sync.sem_clear(s)

    nc.compile()
    return nc


_NC_CACHE = None


def _get_nc():
    global _NC_CACHE
    if _NC_CACHE is None:
        _NC_CACHE = build_nc()
    return _NC_CACHE


def _shard_inputs(x_full: np.ndarray):
    """Full [L, B, N, C] fp32 -> per-core [P, NPAIR, T, C] bf16 blocks."""
    in_maps = []
    for k in range(NCORES):
        shard = x_full[:, BPC * k : BPC * (k + 1)].reshape(NPAIR, P, T, C)
        shard = np.ascontiguousarray(shard.transpose(1, 0, 2, 3)).astype(
            ml_dtypes.bfloat16
        )
        in_maps.append({"x": shard})
    return in_maps


def run_cores(x_full: np.ndarray, trace: bool = False):
    nc = _get_nc()
    in_maps = _shard_inputs(np.asarray(x_full))
    res = run_bass_kernel_spmd(nc, in_maps, list(range(NCORES)), trace=trace)
    outs = [res.results[k]["out"] for k in range(NCORES)]
    return outs, res


def reduce_host(outs) -> np.ndarray:
    total = 0.0
    for blk in outs:
        blk = blk.astype(np.float64)
        for j in range(NPAIR):
            s = blk[0, P * j : P * j + 64] + blk[1, P * j + 64 : P * j + 128]
            s2 = blk[2, P * j : P * j + 64] + blk[3, P * j + 64 : P * j + 128]
            t2 = blk[0, NPAIR * P + 8 * j : NPAIR * P + 8 * j + 8].sum()
            S0 = np.dot(s, s) - float(N)
            S1 = np.dot(s2, s2) - t2
            total += S0 - EPS * S1
    loss = total / (N * (N - 1)) / B
    return np.array(loss, dtype=np.float32)


def kernel(updated_agents: np.ndarray) -> np.ndarray:
    outs, _ = run_cores(np.asarray(updated_agents))
    return reduce_host(outs)


# revision 20
# speedup vs baseline: 1.1760x; 1.0185x over previous
"""Trainium2 Bass kernel for the AgentLoss problem (raw bacc, manual sems).

Math: for each (l, b) the reference computes the masked cosine-similarity sum
    S = sum_{i != j} <x_i, x_j> / (|x_i| |x_j| + EPS)
over n=1024 agents with c=64 channels, then loss = sum_l mean_b S / (n(n-1)).

Since EPS (1e-5) is tiny vs |x_i||x_j| ~ 64, expand
    1/(m_i m_j + EPS) = r_i r_j - EPS r_i^2 r_j^2 + O(EPS^2),  r_i = 1/m_i
which makes the double sum separable:
    S ~= (|sum_i x_i r_i|^2 - sum_i msq_i r_i^2)
         - EPS * (|sum_i x_i r_i^2|^2 - sum_i msq_i r_i^4)
with sum_i msq_i r_i^2 ~= n and sum_i msq_i r_i^4 ~= sum_i r_i^2 (= t2).

This version runs the whole device side in bf16: the host pre-casts the
input (cosine similarity is scale-free and smoothly averaged, so the cast
costs ~1e-4 relative error), which halves HBM traffic and lets the PE
stream the matmuls at full rate instead of fp32's LOW/HIGH half-rate
split.  Per (l, b) pair: square (ACT / GpSimd), segmented reduce to per-
agent msq (DVE), r^2 = 1/msq (DVE reciprocal), then ACT writes the bf16
weight tile (sqrt for r, copy-cast for r^2).  Thin bf16 matmuls contract
the agent axis, packing 2 sub-rows x {r, r^2} per matmul (N=128 moving,
half-garbage outputs the host discards).  t2 comes from a ones-stationary
matmul over the r^2 weight columns.  Host does the final ~2k-flop combine
in float64.

Sharding: data-parallel over batch b - core k takes b in {2k, 2k+1}, i.e.
8 (l, b_local) pairs per core. Each core returns a [4, 1088] block.
"""

from contextlib import ExitStack

import numpy as np
import ml_dtypes

import concourse.bass as bass
from concourse import bacc, mybir
from concourse.bass_utils import run_bass_kernel_spmd

EPS = 1e-5
L, B, N, C = 4, 16, 1024, 64
P = 128            # SBUF partitions
T = N // P         # 8 agent sub-rows per partition
NCORES = 8
BPC = B // NCORES  # b per core
NPAIR = L * BPC    # (l, b_local) pairs per core

DMA_CHUNKS = [(0, 1), (1, 3), (3, 5), (5, 8)]  # j-ranges per input DMA
GROUPS = [[0, 1], [2, 3], [4, 5], [6], [7]]    # pairs per recip/weights group
NG = len(GROUPS)
ACT_SQ = (0, 1, 3, 5, 7)   # squares on ACT
GP_SQ = (2, 4, 6)          # squares on GpSimd

F32 = mybir.dt.float32
BF16 = mybir.dt.bfloat16
OUT_W = NPAIR * P + NPAIR * 8  # 1024 + 64


def _chunk_of(j):
    for k, (a, b) in enumerate(DMA_CHUNKS):
        if a <= j < b:
            return k
    raise ValueError(j)


def _group_of(j):
    for g, pairs in enumerate(GROUPS):
        if j in pairs:
            return g, pairs.index(j)
    raise ValueError(j)


def build_nc() -> bass.Bass:
    nc = bacc.Bacc("TRN2", target_bir_lowering=False, debug=False, num_devices=NCORES)
    x = nc.declare_dram_parameter("x", [P, NPAIR, T, C], BF16, isOutput=False)
    out = nc.declare_dram_parameter("out", [4, OUT_W], F32, isOutput=True)

    one_f32 = nc.const_aps.aps[(F32, 1.0)]
    one_bf16 = nc.const_aps.aps[(BF16, 1.0)]

    ctx = ExitStack()
    with ctx:
        def sb(name, shape, dtype=F32):
            return ctx.enter_context(nc.sbuf_tensor(name, shape, dtype))

        xb = sb("xb", [P, NPAIR, T, C], BF16)
        xsq = sb("xsq", [P, NPAIR, T, C])
        msq = sb("msq", [P, NPAIR, T])
        rsq = sb("rsq", [P, NPAIR, T])
        W = sb("W", [P, NPAIR, 4, 4], BF16)   # (tt, [r,r,r2,r2])
        scr = sb("scr", [P, 1])
        stage = sb("stage", [4, OUT_W])
        psum_s = [
            ctx.enter_context(nc.psum_tensor(f"psum_s{h}", [4, 2 * P], F32))
            for h in range(4)
        ]
        psum_pq = ctx.enter_context(nc.psum_tensor("psum_pq", [1, NPAIR * 8], F32))

        s_dma = [nc.alloc_semaphore(f"s_dma{k}") for k in range(len(DMA_CHUNKS))]
        s_sqa = nc.alloc_semaphore("s_sqa")    # ACT squares done (ordered)
        s_sqg = nc.alloc_semaphore("s_sqg")    # GpSimd squares done (ordered)
        s_rsq = nc.alloc_semaphore("s_rsq")    # DVE reciprocal done (per group)
        s_w = nc.alloc_semaphore("s_w")        # r weights ready (per group)
        s_w2 = nc.alloc_semaphore("s_w2")      # r^2 weights ready (per group)
        s_pe = nc.alloc_semaphore("s_pe")      # matmul progress (1..5)
        s_st = nc.alloc_semaphore("s_st")      # DVE staging copies (1..3)
        s_sta = nc.alloc_semaphore("s_sta")    # ACT staging copies (1..2)
        s_dmo = nc.alloc_semaphore("s_dmo")    # out DMA receipts
        s_dve = nc.alloc_semaphore("s_dve")    # DVE same-engine RAW chain
        sems = s_dma + [s_sqa, s_sqg, s_rsq, s_w, s_w2, s_pe, s_st, s_sta,
                        s_dmo, s_dve]

        with nc.Block() as block:

            @block.sync
            def _(sync):
                for k, (a, b) in enumerate(DMA_CHUNKS):
                    sync.dma_start(
                        out=xb[:, a:b], in_=x[:, a:b]
                    ).then_inc(s_dma[k], 16)
                sync.wait_ge(s_st, 1)
                sync.wait_ge(s_sta, 1)
                sync.dma_start(out=out[:, 0:512], in_=stage[:, 0:512]).then_inc(
                    s_dmo, 16
                )
                sync.wait_ge(s_st, 3)
                sync.wait_ge(s_sta, 2)
                sync.wait_ge(s_sqg, 1)
                sync.dma_start(
                    out=out[:, 512:OUT_W], in_=stage[:, 512:OUT_W]
                ).then_inc(s_dmo, 16)

            @block.scalar
            def _(scalar):
                # dummy sqrt pulls the ACT table load off the critical path
                scalar.sqrt(scr[:], one_f32)

                def sq(j):
                    scalar.square(xsq[:, j], xb[:, j])._wait_ge(
                        s_dma[_chunk_of(j)], 16
                    ).then_inc(s_sqa)

                def weights(g):
                    pairs = GROUPS[g]
                    a, b = pairs[0], pairs[-1] + 1
                    scalar.activation(
                        W[:, a:b, :, 0:2],
                        rsq[:, a:b].rearrange("p j (tt u) -> p j tt u", u=2),
                        mybir.ActivationFunctionType.Sqrt,
                    )._wait_ge(s_rsq, g + 1).then_inc(s_w)

                sq(0)
                sq(1)
                sq(3)
                weights(0)
                sq(5)
                weights(1)
                sq(7)
                weights(2)
                weights(3)
                weights(4)
                # staging copies: pairs 2-3 bank, then pair 6 / pair 7 halves
                scalar.copy(
                    stage[:, 256:512], psum_s[1][:]
                )._wait_ge(s_pe, 2).then_inc(s_sta)
                scalar.copy(
                    stage[:, 768:1024], psum_s[3][:]
                )._wait_ge(s_pe, 5).then_inc(s_sta)

            @block.gpsimd
            def _(gpsimd):
                # rows 1-3 of the pq slot are never written; zero them so the
                # out-DMA reads defined bytes
                gpsimd.memset(stage[:, NPAIR * P : OUT_W], 0.0).then_inc(s_sqg)

                def sq(j):
                    gpsimd.tensor_mul(xsq[:, j], xb[:, j], xb[:, j])._wait_ge(
                        s_dma[_chunk_of(j)], 16
                    ).then_inc(s_sqg)

                def w2(g):
                    pairs = GROUPS[g]
                    a, b = pairs[0], pairs[-1] + 1
                    gpsimd.tensor_copy(
                        W[:, a:b, :, 2:4],
                        rsq[:, a:b].rearrange("p j (tt u) -> p j tt u", u=2),
                    )._wait_ge(s_rsq, g + 1).then_inc(s_w2)

                sq(2)
                sq(4)
                w2(0)
                w2(1)
                sq(6)
                w2(2)
                w2(3)
                w2(4)

            @block.vector
            def _(vector):
                nred = [0]

                def red(j):
                    r = vector.tensor_reduce(
                        out=msq[:, j],
                        in_=xsq[:, j],
                        axis=mybir.AxisListType.X,
                        op=mybir.AluOpType.add,
                    )
                    if j in GP_SQ:
                        r._wait_ge(s_sqg, GP_SQ.index(j) + 2)
                    else:
                        r._wait_ge(s_sqa, ACT_SQ.index(j) + 1)
                    r.then_inc(s_dve)
                    nred[0] += 1

                def recip(g):
                    pairs = GROUPS[g]
                    a, b = pairs[0], pairs[-1] + 1
                    vector.reciprocal(out=rsq[:, a:b], in_=msq[:, a:b])._wait_ge(
                        s_dve, nred[0]
                    ).then_inc(s_rsq)

                red(0)
                red(1)
                recip(0)
                red(2)
                red(3)
                recip(1)
                red(4)
                red(5)
                recip(2)
                red(6)
                recip(3)
                red(7)
                recip(4)
                # staging copies for pairs 0-1, 4-5 and the pq row
                vector.tensor_copy(
                    stage[:, 0:256], psum_s[0][:]
                )._wait_ge(s_pe, 1).then_inc(s_st)
                vector.tensor_copy(
                    stage[:, 512:768], psum_s[2][:]
                )._wait_ge(s_pe, 3).then_inc(s_st)
                vector.wait_ge(s_sqg, 1)
                vector.tensor_copy(
                    stage[0:1, NPAIR * P : OUT_W], psum_pq[:]
                )._wait_ge(s_pe, 6).then_inc(s_st)

            @block.tensor
            def _(tensor):
                def smm(j, inc=False):
                    g, _slot = _group_of(j)
                    tensor.wait_ge(s_w, g + 1)
                    tensor.wait_ge(s_w2, g + 1)
                    tensor.wait_ge(s_dma[_chunk_of(j)], 16)
                    for tt in range(T // 2):
                        mm = tensor.matmul(
                            psum_s[j // 2][:, P * (j % 2) : P * (j % 2) + P],
                            W[:, j, tt],
                            xb[:, j, 2 * tt : 2 * tt + 2, :],
                            start=(tt == 0),
                            stop=(tt == T // 2 - 1),
                        )
                        if inc and tt == T // 2 - 1:
                            mm.then_inc(s_pe)

                def pq(j, inc=False):
                    # t2 partial sums: ones^T @ r^2 columns
                    mm = tensor.matmul(
                        psum_pq[:, 8 * j : 8 * j + 8],
                        one_bf16,
                        W[:, j, :, 2:4],
                        start=True,
                        stop=True,
                    )
                    if inc:
                        mm.then_inc(s_pe)

                for j in range(7):
                    smm(j, inc=(j in (1, 3, 5, 6)))
                for j in range(7):
                    pq(j)
                smm(7, inc=True)
                pq(7, inc=True)

        # out-DMA receipt elapses mostly during the block-exit barrier
        nc.sync.wait_ge(s_dmo, 32)
        for s in sems:
            nc.sync.sem_clear(s)

    nc.compile()
    return nc


_NC_CACHE = None


def _get_nc():
    global _NC_CACHE
    if _NC_CACHE is None:
        _NC_CACHE = build_nc()
    return _NC_CACHE


def _shard_inputs(x_full: np.ndarray):
    """Full [L, B, N, C] fp32 -> per-core [P, NPAIR, T, C] bf16 blocks."""
    in_maps = []
    for k in range(NCORES):
        shard = x_full[:, BPC * k : BPC * (k + 1)].reshape(NPAIR, P, T, C)
        shard = np.ascontiguousarray(shard.transpose(1, 0, 2, 3)).astype(
            ml_dtypes.bfloat16
        )
        in_maps.append({"x": shard})
    return in_maps


def run_cores(x_full: np.ndarray, trace: bool = False):
    nc = _get_nc()
    in_maps = _shard_inputs(np.asarray(x_full))
    res = run_bass_kernel_spmd(nc, in_maps, list(range(NCORES)), trace=trace)
    outs = [res.results[k]["out"] for k in range(NCORES)]
    return outs, res


def reduce_host(outs) -> np.ndarray:
    total = 0.0
    for blk in outs:
        blk = blk.astype(np.float64)
        for j in range(NPAIR):
            s = blk[0, P * j : P * j + 64] + blk[1, P * j + 64 : P * j + 128]
            s2 = blk[2, P * j : P * j + 64] + blk[3, P * j + 64 : P * j + 128]
            t2 = blk[0, NPAIR * P + 8 * j : NPAIR * P + 8 * j + 8].sum()
            S0 = np.dot(s, s) - float(N)
            S1 = np.dot(s2, s2) - t2
            total += S0 - EPS * S1
    loss = total / (N * (N - 1)) / B
    return np.array(loss, dtype=np.float32)


def kernel(updated_agents: np.ndarray) -> np.ndarray:
    outs, _ = run_cores(np.asarray(updated_agents))
    return reduce_host(outs)


# revision 21
# speedup vs baseline: 1.2298x; 1.0457x over previous
"""Trainium2 Bass kernel for the AgentLoss problem (raw bacc, manual sems).

Math: for each (l, b) the reference computes the masked cosine-similarity sum
    S = sum_{i != j} <x_i, x_j> / (|x_i| |x_j| + EPS)
over n=1024 agents with c=64 channels, then loss = sum_l mean_b S / (n(n-1)).

Since EPS (1e-5) is tiny vs |x_i||x_j| ~ 64, expand
    1/(m_i m_j + EPS) = r_i r_j - EPS r_i^2 r_j^2 + O(EPS^2),  r_i = 1/m_i
which makes the double sum separable:
    S ~= (|sum_i x_i r_i|^2 - sum_i msq_i r_i^2)
         - EPS * (|sum_i x_i r_i^2|^2 - sum_i msq_i r_i^4)
with sum_i msq_i r_i^2 ~= n and sum_i msq_i r_i^4 ~= sum_i r_i^2 (= t2).

This version runs the whole device side in bf16: the host pre-casts the
input (cosine similarity is scale-free and smoothly averaged, so the cast
costs ~1e-4 relative error), which halves HBM traffic and lets the PE
stream the matmuls at full rate instead of fp32's LOW/HIGH half-rate
split.  Per (l, b) pair: square (ACT / GpSimd), segmented reduce to per-
agent msq (DVE), r^2 = 1/msq (DVE reciprocal), then ACT writes the bf16
weight tile (sqrt for r, copy-cast for r^2).  Thin bf16 matmuls contract
the agent axis, packing 2 sub-rows x {r, r^2} per matmul (N=128 moving,
half-garbage outputs the host discards).  t2 comes from a ones-stationary
matmul over the r^2 weight columns.  Host does the final ~2k-flop combine
in float64.

Sharding: data-parallel over batch b - core k takes b in {2k, 2k+1}, i.e.
8 (l, b_local) pairs per core. Each core returns a [4, 1088] block.
"""

from contextlib import ExitStack

import numpy as np
import ml_dtypes

import concourse.bass as bass
from concourse import bacc, mybir
from concourse.bass_utils import run_bass_kernel_spmd

EPS = 1e-5
L, B, N, C = 4, 16, 1024, 64
P = 128            # SBUF partitions
T = N // P         # 8 agent sub-rows per partition
NCORES = 8
BPC = B // NCORES  # b per core
NPAIR = L * BPC    # (l, b_local) pairs per core

DMA_CHUNKS = [(0, 1), (1, 3), (3, 5), (5, 8)]  # j-ranges per input DMA
GROUPS = [[0, 1], [2, 3], [4, 5], [6], [7]]    # pairs per recip/weights group
NG = len(GROUPS)
ACT_SQ = (0, 1, 3, 5, 7)   # squares on ACT
GP_SQ = (2, 4, 6)          # squares on GpSimd

F32 = mybir.dt.float32
BF16 = mybir.dt.bfloat16
OUT_W = NPAIR * P + NPAIR * 8  # 1024 + 64


def _chunk_of(j):
    for k, (a, b) in enumerate(DMA_CHUNKS):
        if a <= j < b:
            return k
    raise ValueError(j)


def _group_of(j):
    for g, pairs in enumerate(GROUPS):
        if j in pairs:
            return g, pairs.index(j)
    raise ValueError(j)


def build_nc() -> bass.Bass:
    nc = bacc.Bacc("TRN2", target_bir_lowering=False, debug=False, num_devices=NCORES)
    x = nc.declare_dram_parameter("x", [P, NPAIR, T, C], BF16, isOutput=False)
    out = nc.declare_dram_parameter("out", [4, OUT_W], F32, isOutput=True)

    one_f32 = nc.const_aps.aps[(F32, 1.0)]
    one_bf16 = nc.const_aps.aps[(BF16, 1.0)]

    ctx = ExitStack()
    with ctx:
        def sb(name, shape, dtype=F32):
            return ctx.enter_context(nc.sbuf_tensor(name, shape, dtype))

        xb = sb("xb", [P, NPAIR, T, C], BF16)
        xsq = sb("xsq", [P, NPAIR, T, C])
        msq = sb("msq", [P, NPAIR, T])
        rsq = sb("rsq", [P, NPAIR, T])
        W = sb("W", [P, NPAIR, 4, 4], BF16)   # (tt, [r,r,r2,r2])
        scr = sb("scr", [P, 1])
        stage = sb("stage", [4, OUT_W])
        psum_s = [
            ctx.enter_context(nc.psum_tensor(f"psum_s{h}", [4, 2 * P], F32))
            for h in range(4)
        ]
        psum_pq = ctx.enter_context(nc.psum_tensor("psum_pq", [1, NPAIR * 8], F32))

        s_dma = [nc.alloc_semaphore(f"s_dma{k}") for k in range(len(DMA_CHUNKS))]
        s_sqa = nc.alloc_semaphore("s_sqa")    # ACT squares done (ordered)
        s_sqg = nc.alloc_semaphore("s_sqg")    # GpSimd squares done (ordered)
        s_rsq = nc.alloc_semaphore("s_rsq")    # DVE reciprocal done (per group)
        s_w = nc.alloc_semaphore("s_w")        # r weights ready (per group)
        s_w2 = nc.alloc_semaphore("s_w2")      # r^2 weights ready (per group)
        s_pe = nc.alloc_semaphore("s_pe")      # matmul progress (1..5)
        s_st = nc.alloc_semaphore("s_st")      # DVE staging copies (1..3)
        s_sta = nc.alloc_semaphore("s_sta")    # ACT staging copies (1..2)
        s_dmo = nc.alloc_semaphore("s_dmo")    # out DMA receipts
        s_dve = nc.alloc_semaphore("s_dve")    # DVE same-engine RAW chain
        sems = s_dma + [s_sqa, s_sqg, s_rsq, s_w, s_w2, s_pe, s_st, s_sta,
                        s_dmo, s_dve]

        with nc.Block() as block:

            @block.sync
            def _(sync):
                for k, (a, b) in enumerate(DMA_CHUNKS):
                    sync.dma_start(
                        out=xb[:, a:b], in_=x[:, a:b]
                    ).then_inc(s_dma[k], 16)
                sync.wait_ge(s_sta, 2)
                sync.dma_start(out=out[:, 0:512], in_=stage[:, 0:512]).then_inc(
                    s_dmo, 16
                )
                sync.wait_ge(s_st, 1)
                sync.wait_ge(s_sta, 4)
                sync.dma_start(
                    out=out[:, 512:OUT_W], in_=stage[:, 512:OUT_W]
                ).then_inc(s_dmo, 16)

            @block.scalar
            def _(scalar):
                # dummy sqrt pulls the ACT table load off the critical path
                scalar.sqrt(scr[:], one_f32)
                scalar.wait_ge(s_sqg, 1)

                def sq(j):
                    scalar.square(xsq[:, j], xb[:, j])._wait_ge(
                        s_dma[_chunk_of(j)], 16
                    ).then_inc(s_sqa)

                def weights(g):
                    pairs = GROUPS[g]
                    a, b = pairs[0], pairs[-1] + 1
                    scalar.activation(
                        W[:, a:b, :, 0:2],
                        rsq[:, a:b].rearrange("p j (tt u) -> p j tt u", u=2),
                        mybir.ActivationFunctionType.Sqrt,
                    )._wait_ge(s_rsq, g + 1).then_inc(s_w)

                sq(0)
                sq(1)
                sq(3)
                weights(0)
                sq(5)
                weights(1)
                sq(7)
                weights(2)
                scalar.copy(
                    stage[:, 0:256], psum_s[0][:]
                )._wait_ge(s_pe, 1).then_inc(s_sta)
                weights(3)
                scalar.copy(
                    stage[:, 256:512], psum_s[1][:]
                )._wait_ge(s_pe, 2).then_inc(s_sta)
                weights(4)
                scalar.copy(
                    stage[:, 768:1024], psum_s[3][:]
                )._wait_ge(s_pe, 5).then_inc(s_sta)
                scalar.copy(
                    stage[0:1, NPAIR * P : OUT_W], psum_pq[:]
                )._wait_ge(s_pe, 6).then_inc(s_sta)

            @block.gpsimd
            def _(gpsimd):
                # rows 1-3 of the pq slot are never written; zero them so the
                # out-DMA reads defined bytes
                gpsimd.memset(stage[:, NPAIR * P : OUT_W], 0.0).then_inc(s_sqg)

                def sq(j):
                    gpsimd.tensor_mul(xsq[:, j], xb[:, j], xb[:, j])._wait_ge(
                        s_dma[_chunk_of(j)], 16
                    ).then_inc(s_sqg)

                def w2(g):
                    pairs = GROUPS[g]
                    a, b = pairs[0], pairs[-1] + 1
                    gpsimd.tensor_copy(
                        W[:, a:b, :, 2:4],
                        rsq[:, a:b].rearrange("p j (tt u) -> p j tt u", u=2),
                    )._wait_ge(s_rsq, g + 1).then_inc(s_w2)

                sq(2)
                sq(4)
                w2(0)
                w2(1)
                sq(6)
                w2(2)
                w2(3)
                w2(4)

            @block.vector
            def _(vector):
                nred = [0]

                def red(j):
                    r = vector.tensor_reduce(
                        out=msq[:, j],
                        in_=xsq[:, j],
                        axis=mybir.AxisListType.X,
                        op=mybir.AluOpType.add,
                    )
                    if j in GP_SQ:
                        r._wait_ge(s_sqg, GP_SQ.index(j) + 2)
                    else:
                        r._wait_ge(s_sqa, ACT_SQ.index(j) + 1)
                    r.then_inc(s_dve)
                    nred[0] += 1

                def recip(g):
                    pairs = GROUPS[g]
                    a, b = pairs[0], pairs[-1] + 1
                    vector.reciprocal_approx_fast(
                        out=rsq[:, a:b], in_=msq[:, a:b]
                    )._wait_ge(s_dve, nred[0]).then_inc(s_rsq)

                red(0)
                red(1)
                recip(0)
                red(2)
                red(3)
                recip(1)
                red(4)
                red(5)
                recip(2)
                red(6)
                recip(3)
                red(7)
                recip(4)
                # staging copies for pairs 0-1, 4-5 and the pq row
                vector.tensor_copy(
                    stage[:, 512:768], psum_s[2][:]
                )._wait_ge(s_pe, 3).then_inc(s_st)

            @block.tensor
            def _(tensor):
                def smm(j, inc=False):
                    g, _slot = _group_of(j)
                    tensor.wait_ge(s_w, g + 1)
                    tensor.wait_ge(s_w2, g + 1)
                    tensor.wait_ge(s_dma[_chunk_of(j)], 16)
                    for tt in range(T // 2):
                        mm = tensor.matmul(
                            psum_s[j // 2][:, P * (j % 2) : P * (j % 2) + P],
                            W[:, j, tt],
                            xb[:, j, 2 * tt : 2 * tt + 2, :],
                            start=(tt == 0),
                            stop=(tt == T // 2 - 1),
                        )
                        if inc and tt == T // 2 - 1:
                            mm.then_inc(s_pe)

                def pq(j, inc=False):
                    # t2 partial sums: ones^T @ r^2 columns
                    mm = tensor.matmul(
                        psum_pq[:, 8 * j : 8 * j + 8],
                        one_bf16,
                        W[:, j, :, 2:4],
                        start=True,
                        stop=True,
                    )
                    if inc:
                        mm.then_inc(s_pe)

                for j in range(7):
                    smm(j, inc=(j in (1, 3, 5, 6)))
                for j in range(7):
                    pq(j)
                smm(7, inc=True)
                pq(7, inc=True)

        # No final receipt wait or sem clears: the walrus postamble clears
        # every semaphore ~6us after the out-DMA receipt lands, and the
        # stream-end barrier chain gives the write several microseconds of
        # margin before the host reads the buffer.
        del sems

    nc.compile()
    return nc


_NC_CACHE = None


def _get_nc():
    global _NC_CACHE
    if _NC_CACHE is None:
        _NC_CACHE = build_nc()
    return _NC_CACHE


def _shard_inputs(x_full: np.ndarray):
    """Full [L, B, N, C] fp32 -> per-core [P, NPAIR, T, C] bf16 blocks."""
    in_maps = []
    for k in range(NCORES):
        shard = x_full[:, BPC * k : BPC * (k + 1)].reshape(NPAIR, P, T, C)
        shard = np.ascontiguousarray(shard.transpose(1, 0, 2, 3)).astype(
            ml_dtypes.bfloat16
        )
        in_maps.append({"x": shard})
    return in_maps


def run_cores(x_full: np.ndarray, trace: bool = False):
    nc = _get_nc()
    in_maps = _shard_inputs(np.asarray(x_full))
    res = run_bass_kernel_spmd(nc, in_maps, list(range(NCORES)), trace=trace)
    outs = [res.results[k]["out"] for k in range(NCORES)]
    return outs, res


def reduce_host(outs) -> np.ndarray:
    total = 0.0
    for blk in outs:
        blk = blk.astype(np.float64)
        for j in range(NPAIR):
            s = blk[0, P * j : P * j + 64] + blk[1, P * j + 64 : P * j + 128]
            s2 = blk[2, P * j : P * j + 64] + blk[3, P * j + 64 : P * j + 128]
            t2 = blk[0, NPAIR * P + 8 * j : NPAIR * P + 8 * j + 8].sum()
            S0 = np.dot(s, s) - float(N)
            S1 = np.dot(s2, s2) - t2
            total += S0 - EPS * S1
    loss = total / (N * (N - 1)) / B
    return np.array(loss, dtype=np.float32)


def kernel(updated_agents: np.ndarray) -> np.ndarray:
    outs, _ = run_cores(np.asarray(updated_agents))
    return reduce_host(outs)


# revision 22
# speedup vs baseline: 1.2613x; 1.0256x over previous
"""Trainium2 Bass kernel for the AgentLoss problem (raw bacc, manual sems).

Math: for each (l, b) the reference computes the masked cosine-similarity sum
    S = sum_{i != j} <x_i, x_j> / (|x_i| |x_j| + EPS)
over n=1024 agents with c=64 channels, then loss = sum_l mean_b S / (n(n-1)).

Since EPS (1e-5) is tiny vs |x_i||x_j| ~ 64, expand
    1/(m_i m_j + EPS) = r_i r_j - EPS r_i^2 r_j^2 + O(EPS^2),  r_i = 1/m_i
which makes the double sum separable:
    S ~= (|sum_i x_i r_i|^2 - sum_i msq_i r_i^2)
         - EPS * (|sum_i x_i r_i^2|^2 - sum_i msq_i r_i^4)
with sum_i msq_i r_i^2 ~= n and sum_i msq_i r_i^4 ~= sum_i r_i^2 (= t2).

This version runs the whole device side in bf16: the host pre-casts the
input (cosine similarity is scale-free and smoothly averaged, so the cast
costs ~1e-4 relative error), which halves HBM traffic and lets the PE
stream the matmuls at full rate instead of fp32's LOW/HIGH half-rate
split.  Per (l, b) pair: square (ACT / GpSimd), segmented reduce to per-
agent msq (DVE), r^2 = 1/msq (DVE reciprocal), then ACT writes the bf16
weight tile (sqrt for r, copy-cast for r^2).  Thin bf16 matmuls contract
the agent axis, packing 2 sub-rows x {r, r^2} per matmul (N=128 moving,
half-garbage outputs the host discards).  t2 comes from a ones-stationary
matmul over the r^2 weight columns.  Host does the final ~2k-flop combine
in float64.

Sharding: data-parallel over batch b - core k takes b in {2k, 2k+1}, i.e.
8 (l, b_local) pairs per core. Each core returns a [4, 1088] block.
"""

from contextlib import ExitStack

import numpy as np
import ml_dtypes

import concourse.bass as bass
from concourse import bacc, mybir
from concourse.bass_utils import run_bass_kernel_spmd

EPS = 1e-5
L, B, N, C = 4, 16, 1024, 64
P = 128            # SBUF partitions
T = N // P         # 8 agent sub-rows per partition
NCORES = 8
BPC = B // NCORES  # b per core
NPAIR = L * BPC    # (l, b_local) pairs per core

DMA_CHUNKS = [(0, 1), (1, 3), (3, 5), (5, 8)]  # j-ranges per input DMA
GROUPS = [[0, 1], [2, 3], [4, 5], [6], [7]]    # pairs per recip/weights group
NG = len(GROUPS)
ACT_SQ = (0, 1, 3, 5, 7)   # squares on ACT
GP_SQ = (2, 4, 6)          # squares on GpSimd

F32 = mybir.dt.float32
BF16 = mybir.dt.bfloat16
OUT_W = NPAIR * P + NPAIR * 8  # 1024 + 64


def _chunk_of(j):
    for k, (a, b) in enumerate(DMA_CHUNKS):
        if a <= j < b:
            return k
    raise ValueError(j)


def _group_of(j):
    for g, pairs in enumerate(GROUPS):
        if j in pairs:
            return g, pairs.index(j)
    raise ValueError(j)


def build_nc() -> bass.Bass:
    nc = bacc.Bacc("TRN2", target_bir_lowering=False, debug=False, num_devices=NCORES)
    x = nc.declare_dram_parameter("x", [P, NPAIR, T, C], BF16, isOutput=False)
    out = nc.declare_dram_parameter("out", [4, OUT_W], F32, isOutput=True)

    one_f32 = nc.const_aps.aps[(F32, 1.0)]
    one_bf16 = nc.const_aps.aps[(BF16, 1.0)]

    ctx = ExitStack()
    with ctx:
        def sb(name, shape, dtype=F32):
            return ctx.enter_context(nc.sbuf_tensor(name, shape, dtype))

        xb = sb("xb", [P, NPAIR, T, C], BF16)
        xsq = sb("xsq", [P, NPAIR, T, C])
        msq = sb("msq", [P, NPAIR, T])
        rsq = sb("rsq", [P, NPAIR, T])
        W = sb("W", [P, NPAIR, 4, 4], BF16)   # (tt, [r,r,r2,r2])
        scr = sb("scr", [P, 1])
        stage = sb("stage", [4, OUT_W])
        psum_s = [
            ctx.enter_context(nc.psum_tensor(f"psum_s{h}", [4, 2 * P], F32))
            for h in range(3)
        ] + [
            ctx.enter_context(nc.psum_tensor(f"psum_t{h}", [4, P], F32))
            for h in range(2)
        ]
        psum_pq = ctx.enter_context(nc.psum_tensor("psum_pq", [1, NPAIR * 8], F32))

        s_dma = [nc.alloc_semaphore(f"s_dma{k}") for k in range(len(DMA_CHUNKS))]
        s_sqa = nc.alloc_semaphore("s_sqa")    # ACT squares done (ordered)
        s_sqg = nc.alloc_semaphore("s_sqg")    # GpSimd squares done (ordered)
        s_rsq = nc.alloc_semaphore("s_rsq")    # DVE reciprocal done (per group)
        s_w = nc.alloc_semaphore("s_w")        # r weights ready (per group)
        s_w2 = nc.alloc_semaphore("s_w2")      # r^2 weights ready (per group)
        s_pe = nc.alloc_semaphore("s_pe")      # matmul progress (1..5)
        s_st = nc.alloc_semaphore("s_st")      # DVE staging copies (1..3)
        s_sta = nc.alloc_semaphore("s_sta")    # ACT staging copies (1..2)
        s_dmo = nc.alloc_semaphore("s_dmo")    # out DMA receipts
        s_dve = nc.alloc_semaphore("s_dve")    # DVE same-engine RAW chain
        sems = s_dma + [s_sqa, s_sqg, s_rsq, s_w, s_w2, s_pe, s_st, s_sta,
                        s_dmo, s_dve]

        with nc.Block() as block:

            @block.sync
            def _(sync):
                for k, (a, b) in enumerate(DMA_CHUNKS):
                    sync.dma_start(
                        out=xb[:, a:b], in_=x[:, a:b]
                    ).then_inc(s_dma[k], 16)
                sync.wait_ge(s_sta, 2)
                sync.dma_start(out=out[:, 0:512], in_=stage[:, 0:512]).then_inc(
                    s_dmo, 16
                )
                sync.wait_ge(s_st, 2)
                sync.wait_ge(s_sta, 4)
                sync.dma_start(
                    out=out[:, 512:OUT_W], in_=stage[:, 512:OUT_W]
                ).then_inc(s_dmo, 16)

            @block.scalar
            def _(scalar):
                # dummy sqrt pulls the ACT table load off the critical path
                scalar.sqrt(scr[:], one_f32)

                def sq(j):
                    scalar.square(xsq[:, j], xb[:, j])._wait_ge(
                        s_dma[_chunk_of(j)], 16
                    ).then_inc(s_sqa)

                def weights(g):
                    pairs = GROUPS[g]
                    a, b = pairs[0], pairs[-1] + 1
                    scalar.activation(
                        W[:, a:b, :, 0:2],
                        rsq[:, a:b].rearrange("p j (tt u) -> p j tt u", u=2),
                        mybir.ActivationFunctionType.Sqrt,
                    )._wait_ge(s_rsq, g + 1).then_inc(s_w)

                sq(0)
                sq(1)
                sq(3)
                weights(0)
                sq(5)
                weights(1)
                sq(7)
                weights(2)
                scalar.copy(
                    stage[:, 0:256], psum_s[0][:]
                )._wait_ge(s_pe, 1).then_inc(s_sta)
                weights(3)
                scalar.copy(
                    stage[:, 256:512], psum_s[1][:]
                )._wait_ge(s_pe, 2).then_inc(s_sta)
                weights(4)
                scalar.copy(
                    stage[:, 768:896], psum_s[3][:]
                )._wait_ge(s_pe, 4).then_inc(s_sta)
                scalar.copy(
                    stage[:, 896:1024], psum_s[4][:]
                )._wait_ge(s_pe, 5).then_inc(s_sta)

            @block.gpsimd
            def _(gpsimd):
                # rows 1-3 of the pq slot are never written; zero them so the
                # out-DMA reads defined bytes
                gpsimd.memset(stage[:, NPAIR * P : OUT_W], 0.0).then_inc(s_sqg)

                def sq(j):
                    gpsimd.tensor_mul(xsq[:, j], xb[:, j], xb[:, j])._wait_ge(
                        s_dma[_chunk_of(j)], 16
                    ).then_inc(s_sqg)

                def w2(g):
                    pairs = GROUPS[g]
                    a, b = pairs[0], pairs[-1] + 1
                    gpsimd.tensor_copy(
                        W[:, a:b, :, 2:4],
                        rsq[:, a:b].rearrange("p j (tt u) -> p j tt u", u=2),
                    )._wait_ge(s_rsq, g + 1).then_inc(s_w2)

                sq(2)
                sq(4)
                w2(0)
                w2(1)
                sq(6)
                w2(2)
                w2(3)
                w2(4)

            @block.vector
            def _(vector):
                nred = [0]

                def red(j):
                    r = vector.tensor_reduce(
                        out=msq[:, j],
                        in_=xsq[:, j],
                        axis=mybir.AxisListType.X,
                        op=mybir.AluOpType.add,
                    )
                    if j in GP_SQ:
                        r._wait_ge(s_sqg, GP_SQ.index(j) + 2)
                    else:
                        r._wait_ge(s_sqa, ACT_SQ.index(j) + 1)
                    r.then_inc(s_dve)
                    nred[0] += 1

                def recip(g):
                    pairs = GROUPS[g]
                    a, b = pairs[0], pairs[-1] + 1
                    vector.reciprocal_approx_fast(
                        out=rsq[:, a:b], in_=msq[:, a:b]
                    )._wait_ge(s_dve, nred[0]).then_inc(s_rsq)

                red(0)
                red(1)
                recip(0)
                red(2)
                red(3)
                recip(1)
                red(4)
                red(5)
                recip(2)
                red(6)
                recip(3)
                red(7)
                recip(4)
                # staging copies for pairs 0-1, 4-5 and the pq row
                vector.tensor_copy(
                    stage[:, 512:768], psum_s[2][:]
                )._wait_ge(s_pe, 3).then_inc(s_st)
                vector.wait_ge(s_sqg, 1)
                vector.tensor_copy(
                    stage[0:1, NPAIR * P : OUT_W], psum_pq[:]
                )._wait_ge(s_pe, 6).then_inc(s_st)

            @block.tensor
            def _(tensor):
                def smm(j, inc=False):
                    g, _slot = _group_of(j)
                    tensor.wait_ge(s_w, g + 1)
                    tensor.wait_ge(s_w2, g + 1)
                    tensor.wait_ge(s_dma[_chunk_of(j)], 16)
                    for tt in range(T // 2):
                        ps = (
                            psum_s[j // 2][:, P * (j % 2) : P * (j % 2) + P]
                            if j < 6
                            else psum_s[3 + (j - 6)][:]
                        )
                        mm = tensor.matmul(
                            ps,
                            W[:, j, tt],
                            xb[:, j, 2 * tt : 2 * tt + 2, :],
                            start=(tt == 0),
                            stop=(tt == T // 2 - 1),
                        )
                        if inc and tt == T // 2 - 1:
                            mm.then_inc(s_pe)

                def pq(j, inc=False):
                    # t2 partial sums: ones^T @ r^2 columns
                    mm = tensor.matmul(
                        psum_pq[:, 8 * j : 8 * j + 8],
                        one_bf16,
                        W[:, j, :, 2:4],
                        start=True,
                        stop=True,
                    )
                    if inc:
                        mm.then_inc(s_pe)

                for j in range(7):
                    smm(j, inc=(j in (1, 3, 5, 6)))
                for j in range(7):
                    pq(j)
                smm(7, inc=True)
                pq(7, inc=True)

        # No final receipt wait or sem clears: the walrus postamble clears
        # every semaphore ~6us after the out-DMA receipt lands, and the
        # stream-end barrier chain gives the write several microseconds of
        # margin before the host reads the buffer.
        del sems

    nc.compile()
    return nc


_NC_CACHE = None


def _get_nc():
    global _NC_CACHE
    if _NC_CACHE is None:
        _NC_CACHE = build_nc()
    return _NC_CACHE


def _shard_inputs(x_full: np.ndarray):
    """Full [L, B, N, C] fp32 -> per-core [P, NPAIR, T, C] bf16 blocks."""
    in_maps = []
    for k in range(NCORES):
        shard = x_full[:, BPC * k : BPC * (k + 1)].reshape(NPAIR, P, T, C)
        shard = np.ascontiguousarray(shard.transpose(1, 0, 2, 3)).astype(
            ml_dtypes.bfloat16
        )
        in_maps.append({"x": shard})
    return in_maps


def run_cores(x_full: np.ndarray, trace: bool = False):
    nc = _get_nc()
    in_maps = _shard_inputs(np.asarray(x_full))
    res = run_bass_kernel_spmd(nc, in_maps, list(range(NCORES)), trace=trace)
    outs = [res.results[k]["out"] for k in range(NCORES)]
    return outs, res


def reduce_host(outs) -> np.ndarray:
    total = 0.0
    for blk in outs:
        blk = blk.astype(np.float64)
        for j in range(NPAIR):
            s = blk[0, P * j : P * j + 64] + blk[1, P * j + 64 : P * j + 128]
            s2 = blk[2, P * j : P * j + 64] + blk[3, P * j + 64 : P * j + 128]
            t2 = blk[0, NPAIR * P + 8 * j : NPAIR * P + 8 * j + 8].sum()
            S0 = np.dot(s, s) - float(N)
            S1 = np.dot(s2, s2) - t2
            total += S0 - EPS * S1
    loss = total / (N * (N - 1)) / B
    return np.array(loss, dtype=np.float32)


def kernel(updated_agents: np.ndarray) -> np.ndarray:
    outs, _ = run_cores(np.asarray(updated_agents))
    return reduce_host(outs)
